# revision 4
# baseline (speedup 1.0000x reference)
"""Catmull-Rom 4D spline interpolation kernel for Trainium2 (8 NeuronCores).

Problem: knots [16,64,128,128,2] f32, idx [262144,3] f32 (z,y,x coords),
depth scalar. Output [262144, 2] f32.

Strategy:
  - depth is a scalar -> the D axis collapses host-side to a 4-slab window
    knots[d0:d0+4] with 4 weights wd (Catmull-Rom in depth). Only 32 MiB of
    the 128 MiB table is needed.
  - Data-parallel over the N points axis: shard across 8 cores.
  - Per core: depth-reduce the 4 slabs to V[64,128,128,2] (8 MiB) in DRAM,
    compute per-point spline weights/offsets on DVE, gather 4x4x4x2
    neighborhoods via indirect DMA, weighted-reduce on DVE.
"""
import sys

sys.path.insert(0, "/opt/trn_rl_repo")

import numpy as np

import concourse.mybir as mybir
import concourse.tile as tile_mod
from concourse import bass
from concourse.bass import Bass
from concourse.bacc import Bacc
from concourse.tile import TileContext
from concourse import bass_utils

# ---------------------------------------------------------------------------
# Workaround: this walrus build allows 1 sync wait per instruction (2 on
# InstEventSemaphore), but TileContext's tail drain carries one wait per DMA
# sem lane. Split the drain's waits onto EventSemaphore instructions.


def _patched_dab(self, tick_clock, wait_clock):
    nc = self.nc
    drain_bi = nc.sync.drain()
    wait_clock.add_sem_waits(
        drain_bi.ins, tile_mod.ScopedClock({None: tick_clock.global_clock})
    )
    si = drain_bi.ins.sync_info
    waits = list(si.on_wait) if si is not None else []
    if len(waits) > 1:
        si.on_wait = []
        bb = nc.cur_bb.bb
        insts = bb.instructions
        assert insts[-1].name == drain_bi.ins.name
        insts.pop()
        for i in range(0, len(waits), 2):
            ev = mybir.InstEventSemaphore(
                name=nc.get_next_instruction_name(), ins=[], outs=[]
            )
            ev.engine = drain_bi.ins.engine
            ev.sync_info = mybir.SyncInfo(on_wait=waits[i : i + 2], on_update=[])
            nc.register_instruction(ev)
            bb.add_instruction(ev)
        bb.add_instruction(drain_bi.ins)
    nc.all_engine_barrier()
    assert self.sems is not None
    popped = nc._tile_sem_poison_stack.pop()
    assert popped is self._sem_poison
    nc.clear_and_free_semaphores(list(self.sems.allocated().values()))
    nc.all_engine_barrier()


tile_mod.TileContext._drain_and_barrier = _patched_dab

# ---------------------------------------------------------------------------
D, Z, Y, X, C = 16, 64, 128, 128, 2
N = 262144
NCORES = 8
NP = N // NCORES  # 32768 points per core
P = 128
T = 64  # points per partition per super-tile
NST = NP // (P * T)  # 4 super-tiles per core

f32 = mybir.dt.float32
i32 = mybir.dt.int32
AluOp = mybir.AluOpType

# Catmull-Rom uniform basis: weights = [s^3, s^2, s, 1] @ BASIS
_HERMITE = np.array(
    [[2, -2, 1, 1], [-3, 3, -2, -1], [0, 0, 1, 0], [1, 0, 0, 0]], dtype=np.float64
)
_CR = np.array(
    [[0, 1, 0, 0], [0, 0, 1, 0], [-0.5, 0, 0.5, 0], [0, -0.5, 0, 0.5]],
    dtype=np.float64,
)
BASIS = (_HERMITE @ _CR).astype(np.float32)  # [4 powers, 4 knots]


def _spline_weights_ops(nc, pool, s_sb, name):
    """s [P, T] f32 -> w [P, T, 4] f32 via Horner per knot."""
    w_sb = pool.tile([P, T, 4], f32, tag=name)
    u1 = pool.tile([P, T], f32, tag=name + "_u")
    for k in range(4):
        b0, b1, b2, b3 = (float(BASIS[j, k]) for j in range(4))
        nc.vector.tensor_scalar(
            out=u1[:], in0=s_sb[:], scalar1=b0, scalar2=b1, op0=AluOp.mult, op1=AluOp.add
        )
        nc.vector.tensor_tensor(out=u1[:], in0=u1[:], in1=s_sb[:], op=AluOp.mult)
        nc.vector.tensor_scalar(
            out=u1[:], in0=u1[:], scalar1=b2, scalar2=None, op0=AluOp.add
        )
        nc.vector.tensor_tensor(out=u1[:], in0=u1[:], in1=s_sb[:], op=AluOp.mult)
        nc.vector.tensor_scalar(
            out=w_sb[:, :, k], in0=u1[:], scalar1=b3, scalar2=None, op0=AluOp.add
        )
    return w_sb


def build_kernel():
    """Per-core kernel. Inputs:
    knots4  [4, Z, Y, X*C] f32   host-sliced depth window
    wd      [P, 4] f32           depth weights replicated across partitions
    coords  [NST, P, T*3] f32    point coords in device layout
    Output: out [NST, P, T*2] f32
    """
    nc = Bacc("TRN2", target_bir_lowering=False, debug=False, num_devices=NCORES)
    knots4 = nc.dram_tensor("knots4", [4, Z, Y, X * C], f32, kind="ExternalInput")
    wd = nc.dram_tensor("wd", [P, 4], f32, kind="ExternalInput")
    coords = nc.dram_tensor("coords", [NST, P, T * 3], f32, kind="ExternalInput")
    out = nc.dram_tensor("out", [NST, P, T * 2], f32, kind="ExternalOutput")
    vrows = nc.dram_tensor("vrows", [Z * Y * X, C], f32, kind="Internal")

    with TileContext(nc) as tc:
        with tc.tile_pool(name="sbuf", bufs=2) as pool, tc.tile_pool(
            name="const", bufs=1
        ) as cpool:
            # ---- constants
            wd_sb = cpool.tile([P, 4], f32)
            nc.sync.dma_start(out=wd_sb[:], in_=wd[:])
            dzdy_sb = cpool.tile([P, 16], i32)
            nc.gpsimd.iota(
                dzdy_sb[:], pattern=[[Y * X, 4], [X, 4]], base=0, channel_multiplier=0
            )

            # ---- phase A: depth reduction (8 z-slabs per group)
            ZB = 8
            for zg in range(Z // ZB):
                slabs = pool.tile([P, 4, ZB, X * C], f32, tag="slabs")
                for d in range(4):
                    nc.sync.dma_start(
                        out=slabs[:, d, :, :],
                        in_=knots4[d, zg * ZB : (zg + 1) * ZB, :, :].rearrange(
                            "z y f -> y z f"
                        ),
                    )
                acc = pool.tile([P, ZB, X * C], f32, tag="acc")
                nc.vector.tensor_scalar(
                    out=acc[:],
                    in0=slabs[:, 0, :, :],
                    scalar1=wd_sb[:, 0:1],
                    scalar2=None,
                    op0=AluOp.mult,
                )
                for d in range(1, 4):
                    nc.vector.scalar_tensor_tensor(
                        out=acc[:],
                        in0=slabs[:, d, :, :],
                        scalar=wd_sb[:, d : d + 1],
                        in1=acc[:],
                        op0=AluOp.mult,
                        op1=AluOp.add,
                    )
                nc.sync.dma_start(
                    out=vrows[:, :]
                    .rearrange("(z y x) c -> z y (x c)", z=Z, y=Y, x=X)[
                        zg * ZB : (zg + 1) * ZB, :, :
                    ]
                    .rearrange("z y f -> y z f"),
                    in_=acc[:],
                )

            # ---- phase B: per super-tile
            for st in range(NST):
                co = pool.tile([P, T, 3], f32, tag="coords")
                nc.sync.dma_start(
                    out=co[:].rearrange("p t c -> p (t c)"), in_=coords[st, :, :]
                )
                ii = {}
                ww = {}
                dims = {"z": Z, "y": Y, "x": X}
                for a, aname in enumerate("zyx"):
                    ca = pool.tile([P, T], f32, tag="c" + aname)
                    nc.vector.tensor_copy(out=ca[:], in_=co[:, :, a])
                    # i0 = clamp(round_to_nearest(coord - 0.5), 1, dim-3):
                    # the HW f32->i32 cast rounds to nearest, so round(x-0.5)
                    # == floor(x) for non-integer x; integer x gives x or x-1,
                    # both of which interpolate exactly (s=0 or s=1).
                    ch = pool.tile([P, T], f32, tag="ch" + aname)
                    nc.vector.tensor_scalar(
                        out=ch[:], in0=ca[:], scalar1=-0.5, scalar2=None, op0=AluOp.add
                    )
                    ia = pool.tile([P, T], i32, tag="i" + aname)
                    nc.vector.tensor_copy(out=ia[:], in_=ch[:])
                    nc.vector.tensor_scalar(
                        out=ia[:],
                        in0=ia[:],
                        scalar1=1,
                        scalar2=dims[aname] - 3,
                        op0=AluOp.max,
                        op1=AluOp.min,
                    )
                    iaf = pool.tile([P, T], f32, tag="if" + aname)
                    nc.vector.tensor_copy(out=iaf[:], in_=ia[:])
                    sa = pool.tile([P, T], f32, tag="s" + aname)
                    nc.vector.tensor_tensor(
                        out=sa[:], in0=ca[:], in1=iaf[:], op=AluOp.subtract
                    )
                    ii[aname] = ia
                    ww[aname] = _spline_weights_ops(nc, pool, sa, "w" + aname)

                # row base = (iz-1)*Y*X + (iy-1)*X + (ix-1)
                base = pool.tile([P, T], i32, tag="base")
                nc.vector.tensor_scalar(
                    out=base[:],
                    in0=ii["z"][:],
                    scalar1=Y * X,
                    scalar2=-(Y * X + X + 1),
                    op0=AluOp.mult,
                    op1=AluOp.add,
                )
                nc.vector.scalar_tensor_tensor(
                    out=base[:],
                    in0=ii["y"][:],
                    scalar=X,
                    in1=base[:],
                    op0=AluOp.mult,
                    op1=AluOp.add,
                )
                nc.vector.tensor_tensor(
                    out=base[:], in0=base[:], in1=ii["x"][:], op=AluOp.add
                )
                offs = pool.tile([P, T, 16], i32, tag="offs")
                nc.vector.tensor_tensor(
                    out=offs[:],
                    in0=base[:].to_broadcast([P, T, 16]),
                    in1=dzdy_sb[:]
                    .rearrange("p (a k) -> p a k", a=1)
                    .to_broadcast([P, T, 16]),
                    op=AluOp.add,
                )

                # gather: [P, T, 16, 8]; one indirect DMA per (t, zy) column
                g = pool.tile([P, T, 16, 8], f32, tag="g")
                for t in range(T):
                    for q in range(16):
                        nc.gpsimd.indirect_dma_start(
                            out=g[:, t, q, :],
                            out_offset=None,
                            in_=vrows[:],
                            in_offset=bass.IndirectOffsetOnAxis(
                                ap=offs[:, t, q : q + 1], axis=0
                            ),
                        )

                # wzy[p,t,kz,ky] = wz[p,t,kz] * wy[p,t,ky]
                wzy = pool.tile([P, T, 4, 4], f32, tag="wzy")
                nc.vector.tensor_tensor(
                    out=wzy[:],
                    in0=ww["z"][:]
                    .rearrange("p t (k a) -> p t k a", a=1)
                    .to_broadcast([P, T, 4, 4]),
                    in1=ww["y"][:]
                    .rearrange("p t (a k) -> p t a k", a=1)
                    .to_broadcast([P, T, 4, 4]),
                    op=AluOp.mult,
                )
                # P1 = g * wzy (broadcast over the 8 inner elems)
                nc.vector.tensor_tensor(
                    out=g[:],
                    in0=g[:],
                    in1=wzy[:]
                    .rearrange("p t kz ky -> p t (kz ky)")
                    .rearrange("p t (q a) -> p t q a", a=1)
                    .to_broadcast([P, T, 16, 8]),
                    op=AluOp.mult,
                )
                # P2 = P1 * wx (per c slice to keep APs <= 3 free dims)
                gv = g[:].rearrange("p t q (x c) -> p t q x c", x=4, c=2)
                for c in range(2):
                    nc.vector.tensor_tensor(
                        out=gv[:, :, :, :, c],
                        in0=gv[:, :, :, :, c],
                        in1=ww["x"][:]
                        .rearrange("p t (a k) -> p t a k", a=1)
                        .to_broadcast([P, T, 16, 4]),
                        op=AluOp.mult,
                    )
                # reduce over x: view [P, (T q), c, x] -> r1 [P, T*16, 2]
                r1 = pool.tile([P, T * 16, 2], f32, tag="r1")
                nc.vector.tensor_reduce(
                    out=r1[:],
                    in_=g[:].rearrange("p t q (x c) -> p (t q) c x", x=4, c=2),
                    axis=mybir.AxisListType.X,
                    op=AluOp.add,
                )
                # reduce over zy: view [P, T, c, q] -> out [P, T, 2]
                out_sb = pool.tile([P, T, 2], f32, tag="outsb")
                nc.vector.tensor_reduce(
                    out=out_sb[:],
                    in_=r1[:].rearrange("p (t q) c -> p t c q", t=T, q=16),
                    axis=mybir.AxisListType.X,
                    op=AluOp.add,
                )
                nc.sync.dma_start(
                    out=out[st, :, :], in_=out_sb[:].rearrange("p t c -> p (t c)")
                )
    nc.compile()
    return nc


# ---------------------------------------------------------------------------
_BUILT = None


def _get_built():
    global _BUILT
    if _BUILT is None:
        _BUILT = build_kernel()
    return _BUILT


def _host_prep(idx, knots, depth):
    """Compute depth window + weights, per-core input maps."""
    depth = float(depth)
    # searchsorted semantics of the reference: depths = arange(1, 17)
    ind = int(np.searchsorted(np.arange(1, D + 1, dtype=np.float64), depth, side="right"))
    ind = max(1, min(ind, D - 1))
    r = depth - float(ind)  # depths[ind-1] = ind
    dcoord = (ind - 1) + r
    i0 = int(np.floor(dcoord))
    sd = dcoord - i0
    idp = np.clip(i0 - 1 + np.arange(4), 0, D - 1)
    powers = np.array([sd**3, sd**2, sd, 1.0], dtype=np.float64)
    wd = (powers @ BASIS.astype(np.float64)).astype(np.float32)  # [4]

    knots4 = np.ascontiguousarray(knots[idp]).reshape(4, Z, Y, X * C)
    wd_rep = np.tile(wd[None, :], (P, 1))

    in_maps = []
    for core in range(NCORES):
        pts = idx[core * NP : (core + 1) * NP]  # [NP, 3]
        co = np.ascontiguousarray(
            pts.reshape(NST, P, T, 3).reshape(NST, P, T * 3)
        ).astype(np.float32)
        in_maps.append({"knots4": knots4, "wd": wd_rep, "coords": co})
    return in_maps


def kernel(idx, knots, depth):
    idx = np.asarray(idx, dtype=np.float32)
    knots = np.asarray(knots, dtype=np.float32)
    nc = _get_built()
    in_maps = _host_prep(idx, knots, depth)
    res = bass_utils.run_bass_kernel_spmd(nc, in_maps, core_ids=list(range(NCORES)))
    outs = []
    for core in range(NCORES):
        o = res.results[core]["out"].reshape(NST, P, T, 2).reshape(NP, 2)
        outs.append(o)
    return np.concatenate(outs, axis=0)


if __name__ == "__main__":
    nc = build_kernel()
    print("built ok")


# revision 8
# speedup vs baseline: 18470.1468x; 18470.1468x over previous
"""Catmull-Rom 4D spline interpolation kernel for Trainium2 (8 NeuronCores).

Problem: knots [16,64,128,128,2] f32, idx [262144,3] f32 (z,y,x coords),
depth scalar -> out [262144, 2] f32.

Strategy (v2):
  - depth is a scalar -> the D axis collapses host-side to a 4-slab window
    knots[d0:d0+4] with 4 Catmull-Rom depth weights wd.
  - Shard the N points across 8 cores BY SPATIAL z-RANGE (points sorted by
    their z cell host-side, split into 8 equal chunks). Each core only needs
    a 12-slab z-window of the volume.
  - Per core: depth-reduce its 12-slab window to V12 (SBUF), then expand to
    W2[az, ay, ax, jz, jy, c] = sum_{kz,ky} B[jz,kz] B[jy,ky] V[az+kz, ay+ky, ax, c]
    in DRAM (the z/y spline bases folded in as polynomial coefficients).
    A point's whole 4x4x4x2 stencil then reduces to ONE contiguous 512B
    chunk: W2[az, ay, ax..ax+3, :, :, :], gathered with one DMA descriptor
    per point (128 points per indirect DMA).
  - Final reduce on DVE: out[c] = sum_{kx,jz,jy} cx[kx]*sz^jz*sy^jy * chunk.
"""
import sys

sys.path.insert(0, "/opt/trn_rl_repo")

import numpy as np

import concourse.mybir as mybir
import concourse.tile as tile_mod
from concourse import bass
from concourse.bacc import Bacc
from concourse.tile import TileContext
from concourse import bass_utils

# ---------------------------------------------------------------------------
# Workaround: this walrus build allows 1 sync wait per instruction (2 on
# InstEventSemaphore), but TileContext's tail drain carries one wait per DMA
# sem lane. Split the drain's waits onto EventSemaphore instructions.


def _patched_dab(self, tick_clock, wait_clock):
    nc = self.nc
    drain_bi = nc.sync.drain()
    wait_clock.add_sem_waits(
        drain_bi.ins, tile_mod.ScopedClock({None: tick_clock.global_clock})
    )
    si = drain_bi.ins.sync_info
    waits = list(si.on_wait) if si is not None else []
    if len(waits) > 1:
        si.on_wait = []
        bb = nc.cur_bb.bb
        insts = bb.instructions
        assert insts[-1].name == drain_bi.ins.name
        insts.pop()
        for i in range(0, len(waits), 2):
            ev = mybir.InstEventSemaphore(
                name=nc.get_next_instruction_name(), ins=[], outs=[]
            )
            ev.engine = drain_bi.ins.engine
            ev.sync_info = mybir.SyncInfo(on_wait=waits[i : i + 2], on_update=[])
            nc.register_instruction(ev)
            bb.add_instruction(ev)
        bb.add_instruction(drain_bi.ins)
    nc.all_engine_barrier()
    assert self.sems is not None
    popped = nc._tile_sem_poison_stack.pop()
    assert popped is self._sem_poison
    nc.clear_and_free_semaphores(list(self.sems.allocated().values()))
    nc.all_engine_barrier()


tile_mod.TileContext._drain_and_barrier = _patched_dab

# ---------------------------------------------------------------------------
D, Z, Y, X, C = 16, 64, 128, 128, 2
N = 262144
NCORES = 8
NP = N // NCORES  # 32768 points per core
P = 128
T = 64  # points per partition per super-tile
NST = NP // (P * T)  # 4 super-tiles per core
ZW = 13  # z-slab window per core
AZ = 10  # az = iz-1 in [0, 9]

f32 = mybir.dt.float32
i32 = mybir.dt.int32
AluOp = mybir.AluOpType

# Catmull-Rom uniform basis: weights = [s^3, s^2, s, 1] @ BASIS
_HERMITE = np.array(
    [[2, -2, 1, 1], [-3, 3, -2, -1], [0, 0, 1, 0], [1, 0, 0, 0]], dtype=np.float64
)
_CR = np.array(
    [[0, 1, 0, 0], [0, 0, 1, 0], [-0.5, 0, 0.5, 0], [0, -0.5, 0, 0.5]],
    dtype=np.float64,
)
BASIS = (_HERMITE @ _CR).astype(np.float32)  # [4 powers (s^3..s^0), 4 knots]
# BB[j, k]: weight of s^j for knot k
BB = BASIS[::-1].copy()  # rows now s^0, s^1, s^2, s^3


def build_kernel():
    """Per-core kernel (SPMD; per-core data differs). Inputs:
    knots12 [4, ZW, Y, X*C] f32  host-sliced depth+z window
    wd      [P, 4] f32           depth weights replicated across partitions
    coords  [NST, P, T*3] f32    z-rebased coords in device layout
    Output: out [NST, P, T*2] f32
    """
    nc = Bacc("TRN2", target_bir_lowering=False, debug=False, num_devices=NCORES)
    knots12 = nc.dram_tensor("knots12", [4, ZW, Y, X * C], f32, kind="ExternalInput")
    wd = nc.dram_tensor("wd", [P, 4], f32, kind="ExternalInput")
    coords = nc.dram_tensor("coords", [NST, P, T * 3], f32, kind="ExternalInput")
    out = nc.dram_tensor("out", [NST, P, T * 2], f32, kind="ExternalOutput")
    # W2 rows: ((az*128 + ay)*128 + ax) -> 32 f32 (jz, jy, c)
    w2rows = nc.dram_tensor("w2rows", [AZ * Y * X, 32], f32, kind="Internal")

    with TileContext(nc) as tc:
        with tc.tile_pool(name="const", bufs=1) as cpool:
            wd_sb = cpool.tile([P, 4], f32)
            nc.sync.dma_start(out=wd_sb[:], in_=wd[:])
            # V12 [ay-part, z, x, c] stays resident through phase A
            v12 = cpool.tile([P, ZW, X, C], f32)

            # ---- phase A1: load + depth-reduce into V12
            with tc.tile_pool(name="pA", bufs=2) as pa:
                zchunks = [(0, 4), (4, 4), (8, 5)]
                for z0, zn in zchunks:
                    slabs = pa.tile([P, 4, 5, X * C], f32, tag="slabs")
                    for d in range(4):
                        nc.sync.dma_start(
                            out=slabs[:, d, :zn, :],
                            in_=knots12[d, z0 : z0 + zn, :, :].rearrange(
                                "z y f -> y z f"
                            ),
                        )
                    vslice = v12[:, z0 : z0 + zn, :, :].rearrange(
                        "p z x c -> p z (x c)"
                    )
                    nc.vector.tensor_scalar(
                        out=vslice,
                        in0=slabs[:, 0, :zn, :],
                        scalar1=wd_sb[:, 0:1],
                        scalar2=None,
                        op0=AluOp.mult,
                    )
                    for d in range(1, 4):
                        nc.vector.scalar_tensor_tensor(
                            out=vslice,
                            in0=slabs[:, d, :zn, :],
                            scalar=wd_sb[:, d : d + 1],
                            in1=vslice,
                            op0=AluOp.mult,
                            op1=AluOp.add,
                        )

            # ---- phase A2: jy-expansion
            # A[ay-part, z, x, jy, c] = sum_ky BB[jy,ky] * V12[ay+ky, z, x, c]
            # DVE lanes cannot read shifted partitions: make ky-shifted copies
            # of V12 via SBUF->SBUF DMA first.
            v12s = [v12]
            for ky in range(1, 4):
                vk = cpool.tile([P, ZW, X, C], f32, tag=f"v12s{ky}")
                nc.sync.dma_start(out=vk[0 : P - ky, :, :, :], in_=v12[ky:P, :, :, :])
                v12s.append(vk)
            with tc.tile_pool(name="pB", bufs=1) as pb:
                a_sb = pb.tile([P, ZW, X, 4, C], f32)
                NAY = Y - 3  # ay in [0, 124]; build 125 partitions
                nc.vector.memset(a_sb[:], 0.0)
                for jy in range(4):
                    for ky in range(4):
                        b = float(BB[jy, ky])
                        src = v12s[ky][0:NAY, :, :, :]
                        dst = a_sb[0:NAY, :, :, jy, :]
                        if ky == 0:
                            nc.vector.tensor_scalar(
                                out=dst, in0=src, scalar1=b, scalar2=None, op0=AluOp.mult
                            )
                        else:
                            nc.vector.scalar_tensor_tensor(
                                out=dst,
                                in0=src,
                                scalar=b,
                                in1=dst,
                                op0=AluOp.mult,
                                op1=AluOp.add,
                            )

                # ---- phase A3: jz-expansion + store to DRAM, per az
                # W2[az, ay, ax, jz, jy, c] = sum_kz BB[jz,kz] * A[az+kz, ay, ax, jy, c]
                with tc.tile_pool(name="pC", bufs=2) as pc:
                    for az in range(AZ):
                        w2t = pc.tile([P, X, 4, 4, C], f32, tag="w2t")
                        for jz in range(4):
                            for kz in range(4):
                                b = float(BB[jz, kz])
                                src = a_sb[:, az + kz, :, :, :]
                                dst = w2t[:, :, jz, :, :]
                                if kz == 0:
                                    nc.vector.tensor_scalar(
                                        out=dst,
                                        in0=src,
                                        scalar1=b,
                                        scalar2=None,
                                        op0=AluOp.mult,
                                    )
                                else:
                                    nc.vector.scalar_tensor_tensor(
                                        out=dst,
                                        in0=src,
                                        scalar=b,
                                        in1=dst,
                                        op0=AluOp.mult,
                                        op1=AluOp.add,
                                    )
                        # store: row (az*128 + ay)*128 + ax
                        nc.sync.dma_start(
                            out=w2rows[:, :]
                            .rearrange("(az ay ax) f -> az ay (ax f)", az=AZ, ay=Y, ax=X)[
                                az, :, :
                            ],
                            in_=w2t[:].rearrange("p x jz jy c -> p (x jz jy c)"),
                        )

        # ---- phase B: per super-tile gather + reduce
        with tc.tile_pool(name="sbuf", bufs=2) as pool:
            for st in range(NST):
                co = pool.tile([P, T, 3], f32, tag="coords")
                nc.sync.dma_start(
                    out=co[:].rearrange("p t c -> p (t c)"), in_=coords[st, :, :]
                )
                dims = {"z": 10, "y": Y - 3, "x": X - 3}
                ii = {}
                ss = {}
                for a, aname in enumerate("zyx"):
                    ca = pool.tile([P, T], f32, tag="c" + aname)
                    nc.vector.tensor_copy(out=ca[:], in_=co[:, :, a])
                    # i0 = clamp(round_to_nearest(coord - 0.5), 1, hi)
                    ch = pool.tile([P, T], f32, tag="ch" + aname)
                    nc.vector.tensor_scalar(
                        out=ch[:], in0=ca[:], scalar1=-0.5, scalar2=None, op0=AluOp.add
                    )
                    ia = pool.tile([P, T], i32, tag="i" + aname)
                    nc.vector.tensor_copy(out=ia[:], in_=ch[:])
                    nc.vector.tensor_scalar(
                        out=ia[:],
                        in0=ia[:],
                        scalar1=1,
                        scalar2=dims[aname],
                        op0=AluOp.max,
                        op1=AluOp.min,
                    )
                    iaf = pool.tile([P, T], f32, tag="if" + aname)
                    nc.vector.tensor_copy(out=iaf[:], in_=ia[:])
                    sa = pool.tile([P, T], f32, tag="s" + aname)
                    nc.vector.tensor_tensor(
                        out=sa[:], in0=ca[:], in1=iaf[:], op=AluOp.subtract
                    )
                    ii[aname] = ia
                    ss[aname] = sa

                # row base = ((iz-1)*128 + (iy-1))*128 + (ix-1)
                base = pool.tile([P, T], i32, tag="base")
                nc.vector.tensor_scalar(
                    out=base[:],
                    in0=ii["z"][:],
                    scalar1=Y * X,
                    scalar2=-(Y * X + X + 1),
                    op0=AluOp.mult,
                    op1=AluOp.add,
                )
                nc.vector.scalar_tensor_tensor(
                    out=base[:],
                    in0=ii["y"][:],
                    scalar=X,
                    in1=base[:],
                    op0=AluOp.mult,
                    op1=AluOp.add,
                )
                nc.vector.tensor_tensor(
                    out=base[:], in0=base[:], in1=ii["x"][:], op=AluOp.add
                )

                # gather: one 512B descriptor per point
                g = pool.tile([P, T, 128], f32, tag="g")
                for t in range(T):
                    nc.gpsimd.indirect_dma_start(
                        out=g[:, t, :],
                        out_offset=None,
                        in_=w2rows[:],
                        in_offset=bass.IndirectOffsetOnAxis(
                            ap=base[:, t : t + 1], axis=0
                        ),
                    )

                # weights: pz = [1, sz, sz^2, sz^3], py likewise; cx = Horner
                pw = {}
                for aname in "zy":
                    pa_ = pool.tile([P, T, 4], f32, tag="pw" + aname)
                    nc.vector.memset(pa_[:, :, 0], 1.0)
                    nc.vector.tensor_copy(out=pa_[:, :, 1], in_=ss[aname][:])
                    nc.vector.tensor_tensor(
                        out=pa_[:, :, 2],
                        in0=ss[aname][:],
                        in1=ss[aname][:],
                        op=AluOp.mult,
                    )
                    nc.vector.tensor_tensor(
                        out=pa_[:, :, 3],
                        in0=pa_[:, :, 2],
                        in1=ss[aname][:],
                        op=AluOp.mult,
                    )
                    pw[aname] = pa_
                cx = pool.tile([P, T, 4], f32, tag="cx")
                u1 = pool.tile([P, T], f32, tag="cx_u")
                sx = ss["x"]
                for k in range(4):
                    b0, b1, b2, b3 = (float(BASIS[j, k]) for j in range(4))
                    nc.vector.tensor_scalar(
                        out=u1[:], in0=sx[:], scalar1=b0, scalar2=b1,
                        op0=AluOp.mult, op1=AluOp.add,
                    )
                    nc.vector.tensor_tensor(out=u1[:], in0=u1[:], in1=sx[:], op=AluOp.mult)
                    nc.vector.tensor_scalar(
                        out=u1[:], in0=u1[:], scalar1=b2, scalar2=None, op0=AluOp.add
                    )
                    nc.vector.tensor_tensor(out=u1[:], in0=u1[:], in1=sx[:], op=AluOp.mult)
                    nc.vector.tensor_scalar(
                        out=cx[:, :, k], in0=u1[:], scalar1=b3, scalar2=None, op0=AluOp.add
                    )

                # pzy[p,t,jz,jy] = pz[jz]*py[jy]
                pzy = pool.tile([P, T, 4, 4], f32, tag="pzy")
                nc.vector.tensor_tensor(
                    out=pzy[:],
                    in0=pw["z"][:]
                    .rearrange("p t (k a) -> p t k a", a=1)
                    .to_broadcast([P, T, 4, 4]),
                    in1=pw["y"][:]
                    .rearrange("p t (a k) -> p t a k", a=1)
                    .to_broadcast([P, T, 4, 4]),
                    op=AluOp.mult,
                )
                # P1: g[p,t,kx,jzjy,c] *= pzy (bcast over kx via per-kx ops, c split)
                gv = g[:].rearrange("p t (kx q c) -> p t kx q c", kx=4, q=16, c=2)
                pzyf = pzy[:].rearrange("p t a b -> p t (a b)")
                for c in range(2):
                    for kx in range(4):
                        nc.vector.tensor_tensor(
                            out=gv[:, :, kx, :, c],
                            in0=gv[:, :, kx, :, c],
                            in1=pzyf,
                            op=AluOp.mult,
                        )
                # P2: *= cx[kx] (bcast over q, c split)
                for c in range(2):
                    nc.vector.tensor_tensor(
                        out=gv[:, :, :, :, c],
                        in0=gv[:, :, :, :, c],
                        in1=cx[:]
                        .rearrange("p t (k a) -> p t k a", a=1)
                        .to_broadcast([P, T, 4, 16]),
                        op=AluOp.mult,
                    )
                # reduce: sum over (kx, q) keep (t, c)
                r1 = pool.tile([P, T * 4, 2], f32, tag="r1")
                nc.vector.tensor_reduce(
                    out=r1[:],
                    in_=g[:].rearrange("p t (kx q c) -> p (t kx) c q", kx=4, q=16, c=2),
                    axis=mybir.AxisListType.X,
                    op=AluOp.add,
                )
                out_sb = pool.tile([P, T, 2], f32, tag="outsb")
                nc.vector.tensor_reduce(
                    out=out_sb[:],
                    in_=r1[:].rearrange("p (t kx) c -> p t c kx", t=T, kx=4),
                    axis=mybir.AxisListType.X,
                    op=AluOp.add,
                )
                nc.sync.dma_start(
                    out=out[st, :, :], in_=out_sb[:].rearrange("p t c -> p (t c)")
                )
    nc.compile()
    return nc


# ---------------------------------------------------------------------------
_BUILT = None


def _get_built():
    global _BUILT
    if _BUILT is None:
        _BUILT = build_kernel()
    return _BUILT


def _host_prep(idx, knots, depth):
    depth = float(depth)
    ind = int(
        np.searchsorted(np.arange(1, D + 1, dtype=np.float64), depth, side="right")
    )
    ind = max(1, min(ind, D - 1))
    r = depth - float(ind)
    dcoord = (ind - 1) + r
    i0 = int(np.floor(dcoord))
    sd = dcoord - i0
    idp = np.clip(i0 - 1 + np.arange(4), 0, D - 1)
    powers = np.array([sd**3, sd**2, sd, 1.0], dtype=np.float64)
    wdv = (powers @ BASIS.astype(np.float64)).astype(np.float32)
    wd_rep = np.tile(wdv[None, :], (P, 1))
    knots4 = knots[idp]  # [4, Z, Y, X, C] view

    # shard points by z range: sort by device-exact z key
    zkey = np.rint(idx[:, 0].astype(np.float32) - np.float32(0.5)).astype(np.int64)
    zkey = np.clip(zkey, 1, Z - 3)
    perm = np.argsort(zkey, kind="stable")
    in_maps = []
    for core in range(NCORES):
        sel = perm[core * NP : (core + 1) * NP]
        k_lo = int(zkey[sel[0]])
        k_hi = int(zkey[sel[-1]])
        assert k_hi - k_lo <= 9, (k_lo, k_hi)
        slice_start = min(k_lo - 1, Z - ZW)
        kn = np.ascontiguousarray(
            knots4[:, slice_start : slice_start + ZW]
        ).reshape(4, ZW, Y, X * C)
        pts = idx[sel].astype(np.float32).copy()
        pts[:, 0] -= np.float32(slice_start)  # exact for integer shift
        co = np.ascontiguousarray(pts.reshape(NST, P, T, 3).reshape(NST, P, T * 3))
        in_maps.append({"knots12": kn, "wd": wd_rep, "coords": co})
    return in_maps, perm


def kernel(idx, knots, depth):
    idx = np.asarray(idx, dtype=np.float32)
    knots = np.asarray(knots, dtype=np.float32)
    nc = _get_built()
    in_maps, perm = _host_prep(idx, knots, depth)
    res = bass_utils.run_bass_kernel_spmd(nc, in_maps, core_ids=list(range(NCORES)))
    out_full = np.empty((N, 2), np.float32)
    for core in range(NCORES):
        o = res.results[core]["out"].reshape(NP, 2)
        out_full[perm[core * NP : (core + 1) * NP]] = o
    return out_full


if __name__ == "__main__":
    nc = build_kernel()
    print("built ok")
